# revision 12
# baseline (speedup 1.0000x reference)
"""CrossCompress unit kernel for Trainium2, 8-core data parallel.

Reference computation (per batch row b, D=128):
    item_out[b]   = v[b] * (e[b]@w_vv) + e[b] * (v[b]@w_ev) + bias_v
    entity_out[b] = v[b] * (e[b]@w_ve) + e[b] * (v[b]@w_ee) + bias_e

Strategy: pure data parallel over B=16384 rows -> 2048 rows/core, worked
in a transposed layout [D=128 partitions, batch free].  Per super-tile
the four per-row dot products are PE matmuls whose stationary operand is
the (D,1) weight replicated across 128 columns (host-replicated, bf16)
-- one matmul both computes the dots AND broadcasts the result down all
partitions.  v/e stream in as bf16 (halves input DMA, ~4e-3 rel err
against a 2e-2 gate) and the matmuls run at the 1 cycle/row bf16 pump
rate.  Tile sizes [256,512,512,512,256]: a small first tile gets the PE
and DVE started ~2us earlier, a small last tile shortens the
mul->add->bias->DMA drain chain.  The combine is split by engine
capability: DVE does the wide products ([e|v] * two PSUM dot banks at
once; GPSIMD has no PSUM port), GPSIMD the cross-sums, the Scalar
engine the per-partition bias via Identity activation, writing
item/entity interleaved so each tile leaves in one output DMA.  All DMA
issues live on the otherwise idle SP queue (weights first, then inputs,
then outputs) so no compute queue ever stalls on a DMA descriptor.

Walrus CoreV3 codegen accepts only ONE embedded sync wait per
instruction; a post-pass splits any multi-wait instruction into
single-wait NoOps.
"""
import sys
sys.path.insert(0, '/opt/trn_rl_repo')
import numpy as np
import bass_rust
import concourse.bass as bass
import concourse.tile as tile
from concourse import mybir
from concourse.bass_utils import run_bass_kernel_spmd

B, D = 16384, 128
NCORES = 8
RPC = B // NCORES              # rows per core = 2048
NS = [256, 512, 512, 512, 128, 128]  # batch columns per super-tile
OFF = [0, 256, 768, 1280, 1792, 1920]
# input DMA groups: tiles 0-1 land fast to start the pipe, 2-5 stream behind
GRP = [(0, 2), (2, 6)]
assert sum(NS) == RPC

F32 = mybir.dt.float32
BF16 = mybir.dt.bfloat16
IDENT = mybir.ActivationFunctionType.Identity


def _build():
    nc = bass.Bass("TRN2", target_bir_lowering=False, debug=False,
                   num_devices=NCORES)
    # host-replicated bf16 stationary weights: [D, 4, D]
    xw = nc.dram_tensor("xw", [D, 4, D], BF16, kind="ExternalInput").ap()
    xb = nc.dram_tensor("xb", [D, 2], F32, kind="ExternalInput").ap()
    # input stream: per tile [e | v] blocks, bf16: [D, 2*RPC]
    xin = nc.dram_tensor("xin", [D, 2 * RPC], BF16,
                         kind="ExternalInput").ap()
    # output: item/entity interleaved per batch column: [D, RPC, 2]
    out = nc.dram_tensor("out", [D, RPC, 2], F32, kind="ExternalOutput").ap()

    with tile.TileContext(nc) as tc:
        with tc.tile_pool(name="c0", bufs=1) as c0_pool, \
             tc.tile_pool(name="io", bufs=len(GRP)) as io_pool, \
             tc.tile_pool(name="tmp", bufs=2) as tmp_pool, \
             tc.tile_pool(name="ts", bufs=2) as ts_pool, \
             tc.tile_pool(name="o", bufs=2) as o_pool, \
             tc.tile_pool(name="ps", bufs=2, space="PSUM") as ps_pool:

            w_rep = c0_pool.tile([D, 4, D], BF16)
            nc.sync.dma_start(out=w_rep[:], in_=xw[:, :])
            c_sb = c0_pool.tile([D, 2], F32)
            nc.scalar.dma_start(out=c_sb[:], in_=xb[:, :])
            bv_sb = c_sb[:, 0:1]
            be_sb = c_sb[:, 1:2]

            # grouped input DMAs up front on SP (big contiguous rows)
            ve_tiles = []
            for g0, g1 in GRP:
                cols = 2 * (OFF[g1 - 1] + NS[g1 - 1] - OFF[g0])
                g_sb = io_pool.tile([D, cols], BF16, tag=f"g{g0}")
                nc.sync.dma_start(out=g_sb[:],
                                  in_=xin[:, 2 * OFF[g0]:2 * OFF[g0] + cols])
                for st in range(g0, g1):
                    lo = 2 * (OFF[st] - OFF[g0])
                    ve_tiles.append(g_sb[:, lo:lo + 2 * NS[st]])

            for st, N in enumerate(NS):
                ve_sb = ve_tiles[st]       # AP slice [D, 2N] = [e | v]
                e_b = ve_sb[:, 0:N]
                v_b = ve_sb[:, N:2 * N]

                # dot+broadcast matmuls, bf16, into one PSUM tile:
                #   bank0 = v@w_ev (scales e)   bank1 = e@w_vv (scales v)
                #   bank2 = v@w_ee (scales e)   bank3 = e@w_ve (scales v)
                s4 = ps_pool.tile([D, 4, N], F32, tag="s4")
                nc.tensor.matmul(s4[:, 1], w_rep[:, 1], e_b,
                                 start=True, stop=True)
                nc.tensor.matmul(s4[:, 3], w_rep[:, 3], e_b,
                                 start=True, stop=True)
                nc.tensor.matmul(s4[:, 0], w_rep[:, 0], v_b,
                                 start=True, stop=True)
                nc.tensor.matmul(s4[:, 2], w_rep[:, 2], v_b,
                                 start=True, stop=True)

                # wide products on DVE: [e|v] * [bank0|bank1], [bank2|bank3]
                t12 = tmp_pool.tile([D, 2 * N], F32, tag="t12")
                nc.vector.tensor_mul(t12[:], ve_sb[:], s4[:, 0:2])
                t34 = tmp_pool.tile([D, 2 * N], F32, tag="t34")
                nc.vector.tensor_mul(t34[:], ve_sb[:], s4[:, 2:4])

                # cross-sums on GPSIMD (SBUF only)
                ts1 = ts_pool.tile([D, N], F32, tag="ts1")
                nc.gpsimd.tensor_add(ts1[:], t12[:, 0:N], t12[:, N:2 * N])
                ts2 = ts_pool.tile([D, N], F32, tag="ts2")
                nc.gpsimd.tensor_add(ts2[:], t34[:, 0:N], t34[:, N:2 * N])

                # bias via Identity activation, item/entity interleaved
                o_sb = o_pool.tile([D, N, 2], F32, tag="o")
                nc.scalar.activation(o_sb[:, :, 0], ts1[:], IDENT,
                                     bias=bv_sb, scale=1.0)
                nc.scalar.activation(o_sb[:, :, 1], ts2[:], IDENT,
                                     bias=be_sb, scale=1.0)
                nc.sync.dma_start(out=out[:, OFF[st]:OFF[st] + N],
                                  in_=o_sb[:])
    _split_multiwaits(nc)
    return nc


def _split_multiwaits(nc):
    """Split instructions carrying >1 sync wait into single-wait NoOps
    inserted just before them on the same engine queue."""
    n = 0
    for b in nc.m.functions[0].blocks:
        insts = b.instructions
        new = []
        for inst in insts:
            si = inst.sync_info
            if si is not None and si.on_wait and len(si.on_wait) > 1:
                waits = list(si.on_wait)
                for k, w in enumerate(waits[:-1]):
                    nop = mybir.InstNoOp(name=f"{inst.name}-sw{k}",
                                         ins=[], outs=[])
                    nop.engine = inst.engine
                    nop.sync_info = bass_rust.SyncInfo(on_wait=[w],
                                                       on_update=[])
                    nc.register_instruction(nop)
                    new.append(nop)
                    n += 1
                si.on_wait = [waits[-1]]
            new.append(inst)
        insts[:] = new
    return n


_NC = None


def _get_nc():
    global _NC
    if _NC is None:
        _NC = _build()
    return _NC


def _make_in_maps(v, e, w_vv, w_ve, w_ev, w_ee, bias_v, bias_e):
    import ml_dtypes
    bf16 = ml_dtypes.bfloat16

    xw = np.empty((D, 4, D), bf16)
    xw[:, 0, :] = np.repeat(w_ev.reshape(D, 1), D, axis=1)
    xw[:, 1, :] = np.repeat(w_vv.reshape(D, 1), D, axis=1)
    xw[:, 2, :] = np.repeat(w_ee.reshape(D, 1), D, axis=1)
    xw[:, 3, :] = np.repeat(w_ve.reshape(D, 1), D, axis=1)
    xb = np.stack([bias_v.reshape(D), bias_e.reshape(D)],
                  axis=1).astype(np.float32)

    vT = np.ascontiguousarray(v.T).astype(bf16)   # [D, B]
    eT = np.ascontiguousarray(e.T).astype(bf16)
    in_maps = []
    for c in range(NCORES):
        xin = np.empty((D, 2 * RPC), bf16)
        base = c * RPC
        for st, N in enumerate(NS):
            lo = base + OFF[st]
            xin[:, 2 * OFF[st]:2 * OFF[st] + N] = eT[:, lo:lo + N]
            xin[:, 2 * OFF[st] + N:2 * OFF[st] + 2 * N] = vT[:, lo:lo + N]
        in_maps.append({"xw": xw, "xb": xb, "xin": xin})
    return in_maps


def _run(in_maps, trace=False):
    return run_bass_kernel_spmd(_get_nc(), in_maps, list(range(NCORES)),
                                trace=trace)


def kernel(item_embedding, entity_embedding, w_vv, w_ve, w_ev, w_ee,
           bias_v, bias_e, _trace=False, _res_out=None):
    v = np.asarray(item_embedding, np.float32).reshape(B, D)
    e = np.asarray(entity_embedding, np.float32).reshape(B, D)
    in_maps = _make_in_maps(
        v, e,
        np.asarray(w_vv, np.float32), np.asarray(w_ve, np.float32),
        np.asarray(w_ev, np.float32), np.asarray(w_ee, np.float32),
        np.asarray(bias_v, np.float32), np.asarray(bias_e, np.float32))
    res = _run(in_maps, trace=_trace)
    if _res_out is not None:
        _res_out.append(res)
    item = np.empty((B, D, 1), np.float32)
    ent = np.empty((B, D, 1), np.float32)
    for c in range(NCORES):
        o = res.results[c]["out"]            # [D, RPC, 2]
        item[c * RPC:(c + 1) * RPC, :, 0] = o[:, :, 0].T
        ent[c * RPC:(c + 1) * RPC, :, 0] = o[:, :, 1].T
    return (item, ent)


# revision 16
# speedup vs baseline: 1.2113x; 1.2113x over previous
"""CrossCompress unit kernel for Trainium2, 8-core data parallel.

Reference computation (per batch row b, D=128):
    item_out[b]   = v[b] * (e[b]@w_vv) + e[b] * (v[b]@w_ev) + bias_v
    entity_out[b] = v[b] * (e[b]@w_ve) + e[b] * (v[b]@w_ee) + bias_e

Strategy: pure data parallel over B=16384 rows -> 2048 rows/core, worked
in a transposed layout [D=128 partitions, batch free].  Per super-tile
the four per-row dot products are PE matmuls whose stationary operand is
the (D,1) weight replicated across 128 columns (host-replicated, bf16)
-- one matmul both computes the dots AND broadcasts the result down all
partitions.  v/e stream in as bf16 (halves input DMA, ~4e-3 rel err
against a 2e-2 gate) and the matmuls run at the 1 cycle/row bf16 pump
rate.  Tile sizes [256,512,512,512,256]: a small first tile gets the PE
and DVE started ~2us earlier, a small last tile shortens the
mul->add->bias->DMA drain chain.  The combine is split by engine
capability: DVE does the wide products ([e|v] * two PSUM dot banks at
once; GPSIMD has no PSUM port), GPSIMD the cross-sums, the Scalar
engine the per-partition bias via Identity activation, writing
item/entity interleaved so each tile leaves in one output DMA.  All DMA
issues live on the otherwise idle SP queue (weights first, then inputs,
then outputs) so no compute queue ever stalls on a DMA descriptor.

Walrus CoreV3 codegen accepts only ONE embedded sync wait per
instruction; a post-pass splits any multi-wait instruction into
single-wait NoOps.
"""
import sys
sys.path.insert(0, '/opt/trn_rl_repo')
import numpy as np
import bass_rust
import concourse.bass as bass
import concourse.tile as tile
from concourse import mybir
from concourse.bass_utils import run_bass_kernel_spmd

B, D = 16384, 128
NCORES = 8
RPC = B // NCORES              # rows per core = 2048
NS = [256, 512, 512, 512, 128, 128]  # batch columns per super-tile
OFF = [0, 256, 768, 1280, 1792, 1920]
assert sum(NS) == RPC

F32 = mybir.dt.float32
BF16 = mybir.dt.bfloat16
IDENT = mybir.ActivationFunctionType.Identity


def _build():
    nc = bass.Bass("TRN2", target_bir_lowering=False, debug=False,
                   num_devices=NCORES)
    # host-replicated bf16 stationary weights: [D, 4, D]
    xw = nc.dram_tensor("xw", [D, 4, D], BF16, kind="ExternalInput").ap()
    xb = nc.dram_tensor("xb", [D, 2], F32, kind="ExternalInput").ap()
    # input stream: per tile [e | v] blocks, bf16: [D, 2*RPC]
    xin = nc.dram_tensor("xin", [D, 2 * RPC], BF16,
                         kind="ExternalInput").ap()
    # output: item/entity interleaved per batch column: [D, RPC, 2]
    out = nc.dram_tensor("out", [D, RPC, 2], F32, kind="ExternalOutput").ap()

    with tile.TileContext(nc) as tc:
        with tc.tile_pool(name="c0", bufs=1) as c0_pool, \
             tc.tile_pool(name="io", bufs=len(NS)) as io_pool, \
             tc.tile_pool(name="tmp", bufs=4) as tmp_pool, \
             tc.tile_pool(name="ts", bufs=4) as ts_pool, \
             tc.tile_pool(name="o", bufs=len(NS)) as o_pool, \
             tc.tile_pool(name="ps12", bufs=2, space="PSUM") as ps12_pool, \
             tc.tile_pool(name="ps34", bufs=2, space="PSUM") as ps34_pool:

            w_rep = c0_pool.tile([D, 4, D], BF16)
            nc.sync.dma_start(out=w_rep[:], in_=xw[:, :])
            c_sb = c0_pool.tile([D, 2], F32)
            nc.scalar.dma_start(out=c_sb[:], in_=xb[:, :])
            bv_sb = c_sb[:, 0:1]
            be_sb = c_sb[:, 1:2]

            # per-tile input DMAs up front on SP; completions pace the pipe
            ve_tiles = []
            for st, N in enumerate(NS):
                ve_sb = io_pool.tile([D, 2 * N], BF16, tag=f"ve{st}")
                nc.sync.dma_start(out=ve_sb[:],
                                  in_=xin[:, 2 * OFF[st]:2 * OFF[st] + 2 * N])
                ve_tiles.append(ve_sb)

            for st, N in enumerate(NS):
                ve_sb = ve_tiles[st]       # AP slice [D, 2N] = [e | v]
                e_b = ve_sb[:, 0:N]
                v_b = ve_sb[:, N:2 * N]

                # dot+broadcast matmuls, bf16, into two 2-bank PSUM tiles:
                #   s12 = [v@w_ev | e@w_vv] (item)   s34 = [v@w_ee | e@w_ve]
                s12 = ps12_pool.tile([D, 2, N], F32, tag="s12")
                s34 = ps34_pool.tile([D, 2, N], F32, tag="s34")
                nc.tensor.matmul(s12[:, 1], w_rep[:, 1], e_b,
                                 start=True, stop=True)
                nc.tensor.matmul(s34[:, 1], w_rep[:, 3], e_b,
                                 start=True, stop=True)
                nc.tensor.matmul(s12[:, 0], w_rep[:, 0], v_b,
                                 start=True, stop=True)
                nc.tensor.matmul(s34[:, 0], w_rep[:, 2], v_b,
                                 start=True, stop=True)

                # wide products on DVE: [e|v] * [bank0|bank1] per output
                t12 = tmp_pool.tile([D, 2 * N], F32, tag="t12")
                nc.vector.tensor_mul(t12[:], ve_sb[:], s12[:])
                t34 = tmp_pool.tile([D, 2 * N], F32, tag="t34")
                nc.vector.tensor_mul(t34[:], ve_sb[:], s34[:])

                # cross-sums on GPSIMD (SBUF only)
                ts1 = ts_pool.tile([D, N], F32, tag="ts1")
                nc.gpsimd.tensor_add(ts1[:], t12[:, 0:N], t12[:, N:2 * N])
                ts2 = ts_pool.tile([D, N], F32, tag="ts2")
                nc.gpsimd.tensor_add(ts2[:], t34[:, 0:N], t34[:, N:2 * N])

                # bias via Identity activation, item/entity interleaved
                o_sb = o_pool.tile([D, N, 2], F32, tag="o")
                nc.scalar.activation(o_sb[:, :, 0], ts1[:], IDENT,
                                     bias=bv_sb, scale=1.0)
                nc.scalar.activation(o_sb[:, :, 1], ts2[:], IDENT,
                                     bias=be_sb, scale=1.0)
                nc.sync.dma_start(out=out[:, OFF[st]:OFF[st] + N],
                                  in_=o_sb[:])
    _split_multiwaits(nc)
    return nc


def _split_multiwaits(nc):
    """Split instructions carrying >1 sync wait into single-wait NoOps
    inserted just before them on the same engine queue."""
    n = 0
    for b in nc.m.functions[0].blocks:
        insts = b.instructions
        new = []
        for inst in insts:
            si = inst.sync_info
            if si is not None and si.on_wait and len(si.on_wait) > 1:
                waits = list(si.on_wait)
                for k, w in enumerate(waits[:-1]):
                    nop = mybir.InstNoOp(name=f"{inst.name}-sw{k}",
                                         ins=[], outs=[])
                    nop.engine = inst.engine
                    nop.sync_info = bass_rust.SyncInfo(on_wait=[w],
                                                       on_update=[])
                    nc.register_instruction(nop)
                    new.append(nop)
                    n += 1
                si.on_wait = [waits[-1]]
            new.append(inst)
        insts[:] = new
    return n


_NC = None


def _get_nc():
    global _NC
    if _NC is None:
        _NC = _build()
    return _NC


def _make_in_maps(v, e, w_vv, w_ve, w_ev, w_ee, bias_v, bias_e):
    import ml_dtypes
    bf16 = ml_dtypes.bfloat16

    xw = np.empty((D, 4, D), bf16)
    xw[:, 0, :] = np.repeat(w_ev.reshape(D, 1), D, axis=1)
    xw[:, 1, :] = np.repeat(w_vv.reshape(D, 1), D, axis=1)
    xw[:, 2, :] = np.repeat(w_ee.reshape(D, 1), D, axis=1)
    xw[:, 3, :] = np.repeat(w_ve.reshape(D, 1), D, axis=1)
    xb = np.stack([bias_v.reshape(D), bias_e.reshape(D)],
                  axis=1).astype(np.float32)

    vT = np.ascontiguousarray(v.T).astype(bf16)   # [D, B]
    eT = np.ascontiguousarray(e.T).astype(bf16)
    in_maps = []
    for c in range(NCORES):
        xin = np.empty((D, 2 * RPC), bf16)
        base = c * RPC
        for st, N in enumerate(NS):
            lo = base + OFF[st]
            xin[:, 2 * OFF[st]:2 * OFF[st] + N] = eT[:, lo:lo + N]
            xin[:, 2 * OFF[st] + N:2 * OFF[st] + 2 * N] = vT[:, lo:lo + N]
        in_maps.append({"xw": xw, "xb": xb, "xin": xin})
    return in_maps


def _run(in_maps, trace=False):
    return run_bass_kernel_spmd(_get_nc(), in_maps, list(range(NCORES)),
                                trace=trace)


def kernel(item_embedding, entity_embedding, w_vv, w_ve, w_ev, w_ee,
           bias_v, bias_e, _trace=False, _res_out=None):
    v = np.asarray(item_embedding, np.float32).reshape(B, D)
    e = np.asarray(entity_embedding, np.float32).reshape(B, D)
    in_maps = _make_in_maps(
        v, e,
        np.asarray(w_vv, np.float32), np.asarray(w_ve, np.float32),
        np.asarray(w_ev, np.float32), np.asarray(w_ee, np.float32),
        np.asarray(bias_v, np.float32), np.asarray(bias_e, np.float32))
    res = _run(in_maps, trace=_trace)
    if _res_out is not None:
        _res_out.append(res)
    item = np.empty((B, D, 1), np.float32)
    ent = np.empty((B, D, 1), np.float32)
    for c in range(NCORES):
        o = res.results[c]["out"]            # [D, RPC, 2]
        item[c * RPC:(c + 1) * RPC, :, 0] = o[:, :, 0].T
        ent[c * RPC:(c + 1) * RPC, :, 0] = o[:, :, 1].T
    return (item, ent)


# revision 17
# speedup vs baseline: 1.2148x; 1.0029x over previous
"""CrossCompress unit kernel for Trainium2, 8-core data parallel.

Reference computation (per batch row b, D=128):
    item_out[b]   = v[b] * (e[b]@w_vv) + e[b] * (v[b]@w_ev) + bias_v
    entity_out[b] = v[b] * (e[b]@w_ve) + e[b] * (v[b]@w_ee) + bias_e

Strategy: pure data parallel over B=16384 rows -> 2048 rows/core, worked
in a transposed layout [D=128 partitions, batch free].  Per super-tile
the four per-row dot products are PE matmuls whose stationary operand is
the (D,1) weight replicated across 128 columns (host-replicated, bf16)
-- one matmul both computes the dots AND broadcasts the result down all
partitions.  v/e stream in as bf16 (halves input DMA, ~4e-3 rel err
against a 2e-2 gate) and the matmuls run at the 1 cycle/row bf16 pump
rate.  Tile sizes [256,512,512,512,256]: a small first tile gets the PE
and DVE started ~2us earlier, a small last tile shortens the
mul->add->bias->DMA drain chain.  The combine is split by engine
capability: DVE does the wide products ([e|v] * two PSUM dot banks at
once; GPSIMD has no PSUM port), GPSIMD the cross-sums, the Scalar
engine the per-partition bias via Identity activation, writing
item/entity interleaved so each tile leaves in one output DMA.  All DMA
issues live on the otherwise idle SP queue (weights first, then inputs,
then outputs) so no compute queue ever stalls on a DMA descriptor.

Walrus CoreV3 codegen accepts only ONE embedded sync wait per
instruction; a post-pass splits any multi-wait instruction into
single-wait NoOps.
"""
import sys
sys.path.insert(0, '/opt/trn_rl_repo')
import numpy as np
import bass_rust
import concourse.bass as bass
import concourse.tile as tile
from concourse import mybir
from concourse.bass_utils import run_bass_kernel_spmd

B, D = 16384, 128
NCORES = 8
RPC = B // NCORES              # rows per core = 2048
NS = [128, 256, 512, 512, 512, 128]  # batch columns per super-tile
OFF = [0, 128, 384, 896, 1408, 1920]
assert sum(NS) == RPC

F32 = mybir.dt.float32
BF16 = mybir.dt.bfloat16
IDENT = mybir.ActivationFunctionType.Identity


def _build():
    nc = bass.Bass("TRN2", target_bir_lowering=False, debug=False,
                   num_devices=NCORES)
    # bf16 weight columns; replicated across the PE array via a stride-0
    # (broadcast) stationary access pattern -- no replicated DMA needed
    xw = nc.dram_tensor("xw", [D, 4], BF16, kind="ExternalInput").ap()
    xb = nc.dram_tensor("xb", [D, 2], F32, kind="ExternalInput").ap()
    # input stream: per tile [e | v] blocks, bf16: [D, 2*RPC]
    xin = nc.dram_tensor("xin", [D, 2 * RPC], BF16,
                         kind="ExternalInput").ap()
    # output: item/entity interleaved per batch column, bf16: [D, RPC, 2]
    out = nc.dram_tensor("out", [D, RPC, 2], BF16, kind="ExternalOutput").ap()

    with tile.TileContext(nc) as tc:
        with tc.tile_pool(name="c0", bufs=1) as c0_pool, \
             tc.tile_pool(name="io", bufs=len(NS)) as io_pool, \
             tc.tile_pool(name="tmp", bufs=4) as tmp_pool, \
             tc.tile_pool(name="ts", bufs=4) as ts_pool, \
             tc.tile_pool(name="o", bufs=len(NS)) as o_pool, \
             tc.tile_pool(name="ps12", bufs=2, space="PSUM") as ps12_pool, \
             tc.tile_pool(name="ps34", bufs=2, space="PSUM") as ps34_pool:

            w_sb = c0_pool.tile([D, 4], BF16)
            nc.sync.dma_start(out=w_sb[:], in_=xw[:, :])
            w_rep = [w_sb[:, i:i + 1].broadcast_to([D, D]) for i in range(4)]
            c_sb = c0_pool.tile([D, 2], F32)
            nc.scalar.dma_start(out=c_sb[:], in_=xb[:, :])
            bv_sb = c_sb[:, 0:1]
            be_sb = c_sb[:, 1:2]

            # per-tile input DMAs up front on SP; completions pace the pipe
            ve_tiles = []
            for st, N in enumerate(NS):
                ve_sb = io_pool.tile([D, 2 * N], BF16, tag=f"ve{st}")
                nc.sync.dma_start(out=ve_sb[:],
                                  in_=xin[:, 2 * OFF[st]:2 * OFF[st] + 2 * N])
                ve_tiles.append(ve_sb)

            for st, N in enumerate(NS):
                ve_sb = ve_tiles[st]       # AP slice [D, 2N] = [e | v]
                e_b = ve_sb[:, 0:N]
                v_b = ve_sb[:, N:2 * N]

                # dot+broadcast matmuls, bf16, into two 2-bank PSUM tiles:
                #   s12 = [v@w_ev | e@w_vv] (item)   s34 = [v@w_ee | e@w_ve]
                s12 = ps12_pool.tile([D, 2, N], F32, tag="s12")
                s34 = ps34_pool.tile([D, 2, N], F32, tag="s34")
                nc.tensor.matmul(s12[:, 1], w_rep[1], e_b,
                                 start=True, stop=True)
                nc.tensor.matmul(s34[:, 1], w_rep[3], e_b,
                                 start=True, stop=True)
                nc.tensor.matmul(s12[:, 0], w_rep[0], v_b,
                                 start=True, stop=True)
                nc.tensor.matmul(s34[:, 0], w_rep[2], v_b,
                                 start=True, stop=True)

                # wide products on DVE: [e|v] * [bank0|bank1] per output
                t12 = tmp_pool.tile([D, 2 * N], F32, tag="t12")
                nc.vector.tensor_mul(t12[:], ve_sb[:], s12[:])
                t34 = tmp_pool.tile([D, 2 * N], F32, tag="t34")
                nc.vector.tensor_mul(t34[:], ve_sb[:], s34[:])

                # cross-sums on GPSIMD (SBUF only)
                ts1 = ts_pool.tile([D, N], F32, tag="ts1")
                nc.gpsimd.tensor_add(ts1[:], t12[:, 0:N], t12[:, N:2 * N])
                ts2 = ts_pool.tile([D, N], F32, tag="ts2")
                nc.gpsimd.tensor_add(ts2[:], t34[:, 0:N], t34[:, N:2 * N])

                # bias via Identity activation, item/entity interleaved
                o_sb = o_pool.tile([D, N, 2], BF16, tag="o")
                nc.scalar.activation(o_sb[:, :, 0], ts1[:], IDENT,
                                     bias=bv_sb, scale=1.0)
                nc.scalar.activation(o_sb[:, :, 1], ts2[:], IDENT,
                                     bias=be_sb, scale=1.0)
                nc.sync.dma_start(out=out[:, OFF[st]:OFF[st] + N],
                                  in_=o_sb[:])
    _split_multiwaits(nc)
    return nc


def _split_multiwaits(nc):
    """Split instructions carrying >1 sync wait into single-wait NoOps
    inserted just before them on the same engine queue."""
    n = 0
    for b in nc.m.functions[0].blocks:
        insts = b.instructions
        new = []
        for inst in insts:
            si = inst.sync_info
            if si is not None and si.on_wait and len(si.on_wait) > 1:
                waits = list(si.on_wait)
                for k, w in enumerate(waits[:-1]):
                    nop = mybir.InstNoOp(name=f"{inst.name}-sw{k}",
                                         ins=[], outs=[])
                    nop.engine = inst.engine
                    nop.sync_info = bass_rust.SyncInfo(on_wait=[w],
                                                       on_update=[])
                    nc.register_instruction(nop)
                    new.append(nop)
                    n += 1
                si.on_wait = [waits[-1]]
            new.append(inst)
        insts[:] = new
    return n


_NC = None


def _get_nc():
    global _NC
    if _NC is None:
        _NC = _build()
    return _NC


def _make_in_maps(v, e, w_vv, w_ve, w_ev, w_ee, bias_v, bias_e):
    import ml_dtypes
    bf16 = ml_dtypes.bfloat16

    xw = np.stack([w_ev.reshape(D), w_vv.reshape(D),
                   w_ee.reshape(D), w_ve.reshape(D)], axis=1).astype(bf16)
    xb = np.stack([bias_v.reshape(D), bias_e.reshape(D)],
                  axis=1).astype(np.float32)

    vT = np.ascontiguousarray(v.T).astype(bf16)   # [D, B]
    eT = np.ascontiguousarray(e.T).astype(bf16)
    in_maps = []
    for c in range(NCORES):
        xin = np.empty((D, 2 * RPC), bf16)
        base = c * RPC
        for st, N in enumerate(NS):
            lo = base + OFF[st]
            xin[:, 2 * OFF[st]:2 * OFF[st] + N] = eT[:, lo:lo + N]
            xin[:, 2 * OFF[st] + N:2 * OFF[st] + 2 * N] = vT[:, lo:lo + N]
        in_maps.append({"xw": xw, "xb": xb, "xin": xin})
    return in_maps


def _run(in_maps, trace=False):
    return run_bass_kernel_spmd(_get_nc(), in_maps, list(range(NCORES)),
                                trace=trace)


def kernel(item_embedding, entity_embedding, w_vv, w_ve, w_ev, w_ee,
           bias_v, bias_e, _trace=False, _res_out=None):
    v = np.asarray(item_embedding, np.float32).reshape(B, D)
    e = np.asarray(entity_embedding, np.float32).reshape(B, D)
    in_maps = _make_in_maps(
        v, e,
        np.asarray(w_vv, np.float32), np.asarray(w_ve, np.float32),
        np.asarray(w_ev, np.float32), np.asarray(w_ee, np.float32),
        np.asarray(bias_v, np.float32), np.asarray(bias_e, np.float32))
    res = _run(in_maps, trace=_trace)
    if _res_out is not None:
        _res_out.append(res)
    item = np.empty((B, D, 1), np.float32)
    ent = np.empty((B, D, 1), np.float32)
    for c in range(NCORES):
        o = res.results[c]["out"]            # [D, RPC, 2] bf16
        item[c * RPC:(c + 1) * RPC, :, 0] = o[:, :, 0].T.astype(np.float32)
        ent[c * RPC:(c + 1) * RPC, :, 0] = o[:, :, 1].T.astype(np.float32)
    return (item, ent)
